# revision 3
# baseline (speedup 1.0000x reference)
"""Bidirectional 2-layer LSTM (B=64, T=1024, D=H=256) on 8 Trainium2 NeuronCores.

Self-contained kernel: `kernel(**inputs) -> (seq, h_n, c_n)` mirroring the
reference. Two SPMD launches (one per LSTM layer). Within a launch the 8
cores are (direction, batch-quarter): cores 0-3 forward over batch quarters,
cores 4-7 backward (inputs time-reversed on the host). Each core runs a full
LSTM cell for its 16 batch rows over the 1024 steps:

  - xg = x @ Wih.T (+bias) precomputed in 16-token blocks on the tensor
    engine (weights stationary, fp32).
  - the serial recurrence with a latency-optimized step chain:
      PE: psum pre-filled with xg via an identity matmul (start=True, runs
          early in the previous step's window), then 16 bf16 W_hh@h matmuls
          accumulate on top;
      ACT: one sigmoid over all four gates (the g-gate rows of the weights
          are pre-scaled by 2 so tanh(g) = 2*sigma(2g) - 1);
      DVE: c = f*c + i*tanh(g) via fused tensor_scalar + 3 tensor ops;
      ACT: tanh(c);  DVE: h = sigma(o)*tanh(c)  (bf16 for the next matmul,
      fp32 into the output sequence buffer).

Precision: xg path fp32, recurrent matmul bf16 (weights + h), elementwise
fp32 — measured ~1e-3 max relative error per layer against fp32 numpy.
"""
import sys, os, hashlib, shutil
sys.path.insert(0, "/opt/trn_rl_repo")

import numpy as np

import concourse.bass as bass
import concourse.mybir as mybir
import concourse.tile as tile
from concourse import bacc
import concourse.bass_utils as bass_utils
import concourse.bass2jax as bass2jax

F32 = mybir.dt.float32
BF16 = mybir.dt.bfloat16
AF = mybir.ActivationFunctionType
PSUM = bass.MemorySpace.PSUM

B_FULL, T, D, H, NLAYERS = 64, 1024, 256, 256, 2
B = 16           # per-core batch (4 quarters x 2 directions = 8 cores)
TB = 16          # token block for the xg precompute
G4 = 4 * H
NCHUNK = 8
N_CORES = 8
GATE_PERM = [4, 5, 0, 1, 2, 3, 6, 7]  # psum chunk order g0 g1 i0 i1 f0 f1 o0 o1

# ------------------------------------------------------------------
# persistent NEFF cache (walrus compiles are expensive; key on BIR hash)
# ------------------------------------------------------------------
_NEFF_CACHE_DIR = os.path.join(os.environ.get("TMPDIR", "/tmp"), "bass_neff_cache")
_orig_compile_bir_kernel = bass_utils.compile_bir_kernel


def _cached_compile_bir_kernel(bir_json, tmpdir, neff_name="file.neff"):
    os.makedirs(_NEFF_CACHE_DIR, exist_ok=True)
    key = hashlib.sha256(bir_json).hexdigest()[:32]
    cpath = os.path.join(_NEFF_CACHE_DIR, f"{key}.neff")
    dst = os.path.join(tmpdir, neff_name)
    if os.path.exists(cpath):
        shutil.copyfile(cpath, dst)
        return dst
    neff_file = _orig_compile_bir_kernel(bir_json, tmpdir, neff_name)
    try:
        shutil.copyfile(neff_file, cpath + ".tmp")
        os.replace(cpath + ".tmp", cpath)
    except OSError:
        pass
    return neff_file


bass_utils.compile_bir_kernel = _cached_compile_bir_kernel
bass2jax.compile_bir_kernel = _cached_compile_bir_kernel


# ------------------------------------------------------------------
# bass program builder (one LSTM cell layer, per-core batch B)
# ------------------------------------------------------------------

def build_layer(Din, dt_x=F32, dt_h=BF16):
    KD = Din // 128
    NBLK = T // TB
    nc = bacc.Bacc("TRN2", target_bir_lowering=False, debug=False, num_devices=N_CORES)

    xT = nc.dram_tensor("xT", [KD, 128, T, B], dt_x, kind="ExternalInput").ap()
    wih = nc.dram_tensor("wih", [KD, 128, NCHUNK, 128], dt_x, kind="ExternalInput").ap()
    whh = nc.dram_tensor("whh", [2, 128, NCHUNK, 128], dt_h, kind="ExternalInput").ap()
    bias = nc.dram_tensor("bias", [128, NCHUNK], F32, kind="ExternalInput").ap()
    ident = nc.dram_tensor("ident", [128, 128], dt_x, kind="ExternalInput").ap()
    h0 = nc.dram_tensor("h0", [128, 2 * B], dt_h, kind="ExternalInput").ap()
    c0 = nc.dram_tensor("c0", [128, 2 * B], F32, kind="ExternalInput").ap()
    hseq = nc.dram_tensor("hseq", [128, T, 2 * B], F32, kind="ExternalOutput").ap()
    hT = nc.dram_tensor("hT", [128, 2 * B], F32, kind="ExternalOutput").ap()
    cT = nc.dram_tensor("cT", [128, 2 * B], F32, kind="ExternalOutput").ap()

    with tile.TileContext(nc) as tc:
        with tc.tile_pool(name="const", bufs=1) as const_pool, \
             tc.tile_pool(name="state", bufs=1) as state_pool, \
             tc.tile_pool(name="xblk", bufs=3) as x_pool, \
             tc.tile_pool(name="xg", bufs=2) as xg_pool, \
             tc.tile_pool(name="hblk", bufs=3) as h_pool, \
             tc.tile_pool(name="gates", bufs=4) as gate_pool, \
             tc.tile_pool(name="tmp", bufs=4) as tmp_pool, \
             tc.tile_pool(name="xgps", bufs=1, space=PSUM) as xgps_pool, \
             tc.tile_pool(name="recps", bufs=2, space=PSUM) as recps_pool:

            wih_t = const_pool.tile([128, KD, NCHUNK, 128], dt_x)
            for k in range(KD):
                nc.sync.dma_start(out=wih_t[:, k], in_=wih[k])
            whh_t = const_pool.tile([128, 2, NCHUNK, 128], dt_h)
            for k in range(2):
                nc.sync.dma_start(out=whh_t[:, k], in_=whh[k])
            bias_t = const_pool.tile([128, NCHUNK], F32)
            nc.sync.dma_start(out=bias_t, in_=bias)
            ident_t = const_pool.tile([128, 128], dt_x)
            nc.sync.dma_start(out=ident_t, in_=ident)

            c_t = state_pool.tile([128, 2 * B], F32)
            nc.sync.dma_start(out=c_t, in_=c0)
            h_init = state_pool.tile([128, 2 * B], dt_h)
            nc.sync.dma_start(out=h_init, in_=h0)

            h_prev_tile = h_init
            last_hb = None

            for blk in range(NBLK):
                t0 = blk * TB
                xb = x_pool.tile([128, KD, TB * B], dt_x)
                for k in range(KD):
                    nc.sync.dma_start(out=xb[:, k], in_=xT[k, :, t0:t0 + TB, :])

                xg_ps = xgps_pool.tile([128, NCHUNK, TB, B], F32)
                for m in range(NCHUNK):
                    for k in range(KD):
                        nc.tensor.matmul(
                            xg_ps[:, m], wih_t[:, k, m], xb[:, k],
                            start=(k == 0), stop=(k == KD - 1),
                        )
                slab = xg_pool.tile([128, TB, NCHUNK, B], dt_x)
                for m in range(NCHUNK):
                    nc.vector.tensor_scalar(
                        slab[:, :, m, :], xg_ps[:, m], bias_t[:, m:m + 1], None,
                        mybir.AluOpType.add,
                    )

                hb = h_pool.tile([128, TB, 2 * B], F32)

                for ti in range(TB):
                    g_ps = recps_pool.tile([128, NCHUNK * B], F32)
                    nc.tensor.matmul(g_ps, ident_t, slab[:, ti], start=True, stop=False)
                    for m in range(NCHUNK):
                        for k in range(2):
                            nc.tensor.matmul(
                                g_ps[:, m * B:(m + 1) * B],
                                whh_t[:, k, m],
                                h_prev_tile[:, k * B:(k + 1) * B],
                                start=False,
                                stop=(m == NCHUNK - 1 and k == 1),
                            )
                    # psum cols: g 0:2B, i 2B:4B, f 4B:6B, o 6B:8B
                    sg = gate_pool.tile([128, NCHUNK * B], F32, tag="sg")
                    nc.scalar.activation(sg, g_ps, AF.Sigmoid)
                    tg = tmp_pool.tile([128, 2 * B], F32, tag="tg")
                    nc.vector.tensor_scalar(
                        tg, sg[:, 0:2 * B], 2.0, 1.0,
                        mybir.AluOpType.mult, mybir.AluOpType.subtract)
                    p = tmp_pool.tile([128, 2 * B], F32, tag="p")
                    nc.vector.tensor_mul(p, sg[:, 2 * B:4 * B], tg)
                    q = tmp_pool.tile([128, 2 * B], F32, tag="q")
                    nc.vector.tensor_mul(q, sg[:, 4 * B:6 * B], c_t)
                    nc.vector.tensor_add(c_t, p, q)
                    tc_ = tmp_pool.tile([128, 2 * B], F32, tag="tc")
                    nc.scalar.activation(tc_, c_t, AF.Tanh)
                    h_bf = tmp_pool.tile([128, 2 * B], dt_h, tag="hbf")
                    nc.vector.tensor_mul(h_bf, sg[:, 6 * B:8 * B], tc_)
                    nc.vector.tensor_mul(hb[:, ti], sg[:, 6 * B:8 * B], tc_)
                    h_prev_tile = h_bf

                nc.sync.dma_start(out=hseq[:, t0:t0 + TB], in_=hb)
                last_hb = hb

            nc.sync.dma_start(out=hT, in_=last_hb[:, TB - 1])
            nc.sync.dma_start(out=cT, in_=c_t)

    nc.compile()
    return nc


# ------------------------------------------------------------------
# host-side packing
# ------------------------------------------------------------------

def _np_dt(dt_mm):
    if dt_mm == F32:
        return np.float32
    import ml_dtypes
    return ml_dtypes.bfloat16


def pack_weights(Wih, Whh, b, dt_x=F32, dt_h=BF16):
    Din = Wih.shape[1]
    KD = Din // 128
    dtx, dth = _np_dt(dt_x), _np_dt(dt_h)
    wihp = np.empty((KD, 128, NCHUNK, 128), dtype=dtx)
    whhp = np.empty((2, 128, NCHUNK, 128), dtype=dth)
    biasp = np.empty((128, NCHUNK), dtype=np.float32)
    for j, src in enumerate(GATE_PERM):
        rows = slice(src * 128, (src + 1) * 128)
        scale = 2.0 if j < 2 else 1.0   # tanh(g) = 2*sigma(2g) - 1
        for k in range(KD):
            wihp[k, :, j, :] = (Wih[rows, k * 128:(k + 1) * 128].T * scale).astype(dtx)
        for k in range(2):
            whhp[k, :, j, :] = (Whh[rows, k * 128:(k + 1) * 128].T * scale).astype(dth)
        biasp[:, j] = b[rows] * scale
    return wihp, whhp, biasp


def pack_x(x, dt_x=F32):
    Bq, Tl, Din = x.shape
    KD = Din // 128
    xt = np.ascontiguousarray(x.transpose(2, 1, 0)).reshape(KD, 128, Tl, Bq)
    return xt.astype(_np_dt(dt_x))


def pack_state(s, dt=np.float32):
    out = np.ascontiguousarray(np.asarray(s, np.float32).T).reshape(2, 128, B).transpose(1, 0, 2)
    return np.ascontiguousarray(out).reshape(128, 2 * B).astype(dt)


def unpack_hseq(hseq_arr):
    v = np.asarray(hseq_arr, np.float32).reshape(128, T, 2, B)
    return np.ascontiguousarray(v.transpose(3, 1, 2, 0)).reshape(B, T, H)


def unpack_state(s):
    v = np.asarray(s, np.float32).reshape(128, 2, B)
    return np.ascontiguousarray(v.transpose(2, 1, 0)).reshape(B, H)


# ------------------------------------------------------------------
# layer runner
# ------------------------------------------------------------------

_PROGRAM_CACHE = {}
_LAST_IN_MAPS = {}  # dev aid: last per-core input maps per layer width


def _get_program(Din):
    if Din not in _PROGRAM_CACHE:
        _PROGRAM_CACHE[Din] = build_layer(Din)
    return _PROGRAM_CACHE[Din]


def _run_bilayer(seq, h0_f, c0_f, h0_b, c0_b, wf, bf, wb, bb):
    """seq [64, T, Din]; returns (seq_out [64,T,2H] fp32, hT [64,2H], cT [64,2H])."""
    Din = seq.shape[2]
    nc = _get_program(Din)
    wihp_f, whhp_f, biasp_f = pack_weights(wf[0], wf[1], bf)
    wihp_b, whhp_b, biasp_b = pack_weights(wb[0], wb[1], bb)
    ident = np.eye(128, dtype=np.float32)
    dth = _np_dt(BF16)

    in_maps = []
    for core in range(N_CORES):
        fwd = core < 4
        q = core % 4
        rows = slice(q * B, (q + 1) * B)
        x = seq[rows]
        if not fwd:
            x = x[:, ::-1]
        in_maps.append({
            "xT": pack_x(np.ascontiguousarray(x)),
            "wih": wihp_f if fwd else wihp_b,
            "whh": whhp_f if fwd else whhp_b,
            "bias": biasp_f if fwd else biasp_b,
            "ident": ident,
            "h0": pack_state((h0_f if fwd else h0_b)[rows], dth),
            "c0": pack_state((c0_f if fwd else c0_b)[rows]),
        })

    _LAST_IN_MAPS[Din] = in_maps
    res = bass_utils.run_bass_kernel_spmd(nc, in_maps, core_ids=list(range(N_CORES)))

    seq_out = np.empty((B_FULL, T, 2 * H), np.float32)
    hT = np.empty((B_FULL, 2 * H), np.float32)
    cT = np.empty((B_FULL, 2 * H), np.float32)
    for core in range(N_CORES):
        fwd = core < 4
        q = core % 4
        rows = slice(q * B, (q + 1) * B)
        hs = unpack_hseq(res.results[core]["hseq"])
        if fwd:
            seq_out[rows, :, :H] = hs
        else:
            seq_out[rows, :, H:] = hs[:, ::-1]
        hcol = slice(0, H) if fwd else slice(H, 2 * H)
        hT[rows, hcol] = unpack_state(res.results[core]["hT"])
        cT[rows, hcol] = unpack_state(res.results[core]["cT"])
    return seq_out, hT, cT


def kernel(input_seq, h_0, c_0,
           Wih_f0, Whh_f0, b_f0, Wih_b0, Whh_b0, b_b0,
           Wih_f1, Whh_f1, b_f1, Wih_b1, Whh_b1, b_b1):
    input_seq = np.asarray(input_seq, np.float32)
    h_0 = np.asarray(h_0, np.float32)
    c_0 = np.asarray(c_0, np.float32)

    seq1, hT0, cT0 = _run_bilayer(
        input_seq, h_0[0], c_0[0], h_0[2], c_0[2],
        (np.asarray(Wih_f0, np.float32), np.asarray(Whh_f0, np.float32)),
        np.asarray(b_f0, np.float32),
        (np.asarray(Wih_b0, np.float32), np.asarray(Whh_b0, np.float32)),
        np.asarray(b_b0, np.float32))

    seq2, hT1, cT1 = _run_bilayer(
        seq1, h_0[1], c_0[1], h_0[3], c_0[3],
        (np.asarray(Wih_f1, np.float32), np.asarray(Whh_f1, np.float32)),
        np.asarray(b_f1, np.float32),
        (np.asarray(Wih_b1, np.float32), np.asarray(Whh_b1, np.float32)),
        np.asarray(b_b1, np.float32))

    h_n = np.stack([hT0, hT1])
    c_n = np.stack([cT0, cT1])
    return seq2, h_n, c_n


# revision 4
# speedup vs baseline: 16.4013x; 16.4013x over previous
"""Bidirectional 2-layer LSTM (B=64, T=1024, D=H=256) on 8 Trainium2 NeuronCores.

Self-contained kernel: `kernel(**inputs) -> (seq, h_n, c_n)` mirroring the
reference. Two SPMD launches (one per LSTM layer). Within a launch the 8
cores are (direction, batch-quarter): cores 0-3 forward over batch quarters,
cores 4-7 backward (inputs time-reversed on the host). Each core runs a full
LSTM cell for its 16 batch rows over the 1024 steps:

  - xg = x @ Wih.T (+bias) precomputed in 16-token blocks on the tensor
    engine (weights stationary, fp32).
  - the serial recurrence with a latency-optimized step chain:
      PE: psum pre-filled with xg via an identity matmul (start=True, runs
          early in the previous step's window), then 16 bf16 W_hh@h matmuls
          accumulate on top;
      ACT: one sigmoid over all four gates (the g-gate rows of the weights
          are pre-scaled by 2 so tanh(g) = 2*sigma(2g) - 1);
      DVE: c = f*c + i*tanh(g) via fused tensor_scalar + 3 tensor ops;
      ACT: tanh(c);  DVE: h = sigma(o)*tanh(c)  (bf16 for the next matmul,
      fp32 into the output sequence buffer).

Precision: xg path fp32, recurrent matmul bf16 (weights + h), elementwise
fp32 — measured ~1e-3 max relative error per layer against fp32 numpy.
"""
import sys, os, hashlib, shutil
sys.path.insert(0, "/opt/trn_rl_repo")

import numpy as np

import concourse.bass as bass
import concourse.mybir as mybir
import concourse.tile as tile
from concourse import bacc
import concourse.bass_utils as bass_utils
import concourse.bass2jax as bass2jax

F32 = mybir.dt.float32
BF16 = mybir.dt.bfloat16
AF = mybir.ActivationFunctionType
PSUM = bass.MemorySpace.PSUM

B_FULL, T, D, H, NLAYERS = 64, 1024, 256, 256, 2
B = 16           # per-core batch (4 quarters x 2 directions = 8 cores)
TB = 16          # token block for the xg precompute
G4 = 4 * H
NCHUNK = 8
N_CORES = 8
GATE_PERM = [4, 5, 0, 1, 2, 3, 6, 7]  # psum chunk order g0 g1 i0 i1 f0 f1 o0 o1

# ------------------------------------------------------------------
# persistent NEFF cache (walrus compiles are expensive; key on BIR hash)
# ------------------------------------------------------------------
_NEFF_CACHE_DIR = os.path.join(os.environ.get("TMPDIR", "/tmp"), "bass_neff_cache")
_orig_compile_bir_kernel = bass_utils.compile_bir_kernel


def _cached_compile_bir_kernel(bir_json, tmpdir, neff_name="file.neff"):
    os.makedirs(_NEFF_CACHE_DIR, exist_ok=True)
    key = hashlib.sha256(bir_json).hexdigest()[:32]
    cpath = os.path.join(_NEFF_CACHE_DIR, f"{key}.neff")
    dst = os.path.join(tmpdir, neff_name)
    if os.path.exists(cpath):
        shutil.copyfile(cpath, dst)
        return dst
    neff_file = _orig_compile_bir_kernel(bir_json, tmpdir, neff_name)
    try:
        shutil.copyfile(neff_file, cpath + ".tmp")
        os.replace(cpath + ".tmp", cpath)
    except OSError:
        pass
    return neff_file


bass_utils.compile_bir_kernel = _cached_compile_bir_kernel
bass2jax.compile_bir_kernel = _cached_compile_bir_kernel


# ------------------------------------------------------------------
# bass program builder (one LSTM cell layer, per-core batch B)
# ------------------------------------------------------------------

def build_layer(Din, dt_x=F32, dt_h=BF16, reps=1):
    KD = Din // 128
    NBLK = T // TB
    nc = bacc.Bacc("TRN2", target_bir_lowering=False, debug=False, num_devices=N_CORES)

    xT = nc.dram_tensor("xT", [KD, 128, T, B], dt_x, kind="ExternalInput").ap()
    wih = nc.dram_tensor("wih", [KD, 128, NCHUNK, 128], dt_x, kind="ExternalInput").ap()
    whh = nc.dram_tensor("whh", [2, 128, NCHUNK, 128], dt_h, kind="ExternalInput").ap()
    bias = nc.dram_tensor("bias", [128, NCHUNK], F32, kind="ExternalInput").ap()
    ident = nc.dram_tensor("ident", [128, 128], dt_x, kind="ExternalInput").ap()
    h0 = nc.dram_tensor("h0", [128, 2 * B], dt_h, kind="ExternalInput").ap()
    c0 = nc.dram_tensor("c0", [128, 2 * B], F32, kind="ExternalInput").ap()
    hseq = nc.dram_tensor("hseq", [128, T, 2 * B], F32, kind="ExternalOutput").ap()
    hT = nc.dram_tensor("hT", [128, 2 * B], F32, kind="ExternalOutput").ap()
    cT = nc.dram_tensor("cT", [128, 2 * B], F32, kind="ExternalOutput").ap()

    with tile.TileContext(nc) as tc:
        with tc.tile_pool(name="const", bufs=1) as const_pool, \
             tc.tile_pool(name="state", bufs=1) as state_pool, \
             tc.tile_pool(name="xblk", bufs=3) as x_pool, \
             tc.tile_pool(name="xg", bufs=2) as xg_pool, \
             tc.tile_pool(name="hblk", bufs=3) as h_pool, \
             tc.tile_pool(name="gates", bufs=4) as gate_pool, \
             tc.tile_pool(name="tmp", bufs=4) as tmp_pool, \
             tc.tile_pool(name="xgps", bufs=1, space=PSUM) as xgps_pool, \
             tc.tile_pool(name="recps", bufs=2, space=PSUM) as recps_pool:

            wih_t = const_pool.tile([128, KD, NCHUNK, 128], dt_x)
            for k in range(KD):
                nc.sync.dma_start(out=wih_t[:, k], in_=wih[k])
            whh_t = const_pool.tile([128, 2, NCHUNK, 128], dt_h)
            for k in range(2):
                nc.sync.dma_start(out=whh_t[:, k], in_=whh[k])
            bias_t = const_pool.tile([128, NCHUNK], F32)
            nc.sync.dma_start(out=bias_t, in_=bias)
            ident_t = const_pool.tile([128, 128], dt_x)
            nc.sync.dma_start(out=ident_t, in_=ident)

            c_t = state_pool.tile([128, 2 * B], F32)
            nc.sync.dma_start(out=c_t, in_=c0)
            h_init = state_pool.tile([128, 2 * B], dt_h)
            nc.sync.dma_start(out=h_init, in_=h0)

            h_prev_tile = h_init
            last_hb = None

            for blk in range(NBLK * reps):
                t0 = (blk % NBLK) * TB
                xb = x_pool.tile([128, KD, TB * B], dt_x)
                for k in range(KD):
                    nc.sync.dma_start(out=xb[:, k], in_=xT[k, :, t0:t0 + TB, :])

                xg_ps = xgps_pool.tile([128, NCHUNK, TB, B], F32)
                for m in range(NCHUNK):
                    for k in range(KD):
                        nc.tensor.matmul(
                            xg_ps[:, m], wih_t[:, k, m], xb[:, k],
                            start=(k == 0), stop=(k == KD - 1),
                        )
                slab = xg_pool.tile([128, TB, NCHUNK, B], dt_x)
                for m in range(NCHUNK):
                    nc.vector.tensor_scalar(
                        slab[:, :, m, :], xg_ps[:, m], bias_t[:, m:m + 1], None,
                        mybir.AluOpType.add,
                    )

                hb = h_pool.tile([128, TB, 2 * B], F32)

                for ti in range(TB):
                    g_ps = recps_pool.tile([128, NCHUNK * B], F32)
                    nc.tensor.matmul(g_ps, ident_t, slab[:, ti], start=True, stop=False)
                    for m in range(NCHUNK):
                        for k in range(2):
                            nc.tensor.matmul(
                                g_ps[:, m * B:(m + 1) * B],
                                whh_t[:, k, m],
                                h_prev_tile[:, k * B:(k + 1) * B],
                                start=False,
                                stop=(m == NCHUNK - 1 and k == 1),
                            )
                    # psum cols: g 0:2B, i 2B:4B, f 4B:6B, o 6B:8B
                    sg = gate_pool.tile([128, NCHUNK * B], F32, tag="sg")
                    nc.scalar.activation(sg, g_ps, AF.Sigmoid)
                    tg = tmp_pool.tile([128, 2 * B], F32, tag="tg")
                    nc.vector.tensor_scalar(
                        tg, sg[:, 0:2 * B], 2.0, 1.0,
                        mybir.AluOpType.mult, mybir.AluOpType.subtract)
                    p = tmp_pool.tile([128, 2 * B], F32, tag="p")
                    nc.vector.tensor_mul(p, sg[:, 2 * B:4 * B], tg)
                    q = tmp_pool.tile([128, 2 * B], F32, tag="q")
                    nc.vector.tensor_mul(q, sg[:, 4 * B:6 * B], c_t)
                    nc.vector.tensor_add(c_t, p, q)
                    tc_ = tmp_pool.tile([128, 2 * B], F32, tag="tc")
                    nc.scalar.activation(tc_, c_t, AF.Tanh)
                    h_bf = tmp_pool.tile([128, 2 * B], dt_h, tag="hbf")
                    nc.vector.tensor_mul(h_bf, sg[:, 6 * B:8 * B], tc_)
                    nc.vector.tensor_mul(hb[:, ti], sg[:, 6 * B:8 * B], tc_)
                    h_prev_tile = h_bf

                nc.sync.dma_start(out=hseq[:, t0:t0 + TB], in_=hb)
                last_hb = hb

            nc.sync.dma_start(out=hT, in_=last_hb[:, TB - 1])
            nc.sync.dma_start(out=cT, in_=c_t)

    nc.compile()
    return nc


# ------------------------------------------------------------------
# host-side packing
# ------------------------------------------------------------------

def _np_dt(dt_mm):
    if dt_mm == F32:
        return np.float32
    import ml_dtypes
    return ml_dtypes.bfloat16


def pack_weights(Wih, Whh, b, dt_x=F32, dt_h=BF16):
    Din = Wih.shape[1]
    KD = Din // 128
    dtx, dth = _np_dt(dt_x), _np_dt(dt_h)
    wihp = np.empty((KD, 128, NCHUNK, 128), dtype=dtx)
    whhp = np.empty((2, 128, NCHUNK, 128), dtype=dth)
    biasp = np.empty((128, NCHUNK), dtype=np.float32)
    for j, src in enumerate(GATE_PERM):
        rows = slice(src * 128, (src + 1) * 128)
        scale = 2.0 if j < 2 else 1.0   # tanh(g) = 2*sigma(2g) - 1
        for k in range(KD):
            wihp[k, :, j, :] = (Wih[rows, k * 128:(k + 1) * 128].T * scale).astype(dtx)
        for k in range(2):
            whhp[k, :, j, :] = (Whh[rows, k * 128:(k + 1) * 128].T * scale).astype(dth)
        biasp[:, j] = b[rows] * scale
    return wihp, whhp, biasp


def pack_x(x, dt_x=F32):
    Bq, Tl, Din = x.shape
    KD = Din // 128
    xt = np.ascontiguousarray(x.transpose(2, 1, 0)).reshape(KD, 128, Tl, Bq)
    return xt.astype(_np_dt(dt_x))


def pack_state(s, dt=np.float32):
    out = np.ascontiguousarray(np.asarray(s, np.float32).T).reshape(2, 128, B).transpose(1, 0, 2)
    return np.ascontiguousarray(out).reshape(128, 2 * B).astype(dt)


def unpack_hseq(hseq_arr):
    v = np.asarray(hseq_arr, np.float32).reshape(128, T, 2, B)
    return np.ascontiguousarray(v.transpose(3, 1, 2, 0)).reshape(B, T, H)


def unpack_state(s):
    v = np.asarray(s, np.float32).reshape(128, 2, B)
    return np.ascontiguousarray(v.transpose(2, 1, 0)).reshape(B, H)


# ------------------------------------------------------------------
# layer runner
# ------------------------------------------------------------------

_PROGRAM_CACHE = {}
_LAST_IN_MAPS = {}  # dev aid: last per-core input maps per layer width


def _get_program(Din):
    if Din not in _PROGRAM_CACHE:
        _PROGRAM_CACHE[Din] = build_layer(Din)
    return _PROGRAM_CACHE[Din]


def _run_bilayer(seq, h0_f, c0_f, h0_b, c0_b, wf, bf, wb, bb):
    """seq [64, T, Din]; returns (seq_out [64,T,2H] fp32, hT [64,2H], cT [64,2H])."""
    Din = seq.shape[2]
    nc = _get_program(Din)
    wihp_f, whhp_f, biasp_f = pack_weights(wf[0], wf[1], bf)
    wihp_b, whhp_b, biasp_b = pack_weights(wb[0], wb[1], bb)
    ident = np.eye(128, dtype=np.float32)
    dth = _np_dt(BF16)

    in_maps = []
    for core in range(N_CORES):
        fwd = core < 4
        q = core % 4
        rows = slice(q * B, (q + 1) * B)
        x = seq[rows]
        if not fwd:
            x = x[:, ::-1]
        in_maps.append({
            "xT": pack_x(np.ascontiguousarray(x)),
            "wih": wihp_f if fwd else wihp_b,
            "whh": whhp_f if fwd else whhp_b,
            "bias": biasp_f if fwd else biasp_b,
            "ident": ident,
            "h0": pack_state((h0_f if fwd else h0_b)[rows], dth),
            "c0": pack_state((c0_f if fwd else c0_b)[rows]),
        })

    _LAST_IN_MAPS[Din] = in_maps
    res = bass_utils.run_bass_kernel_spmd(nc, in_maps, core_ids=list(range(N_CORES)))

    seq_out = np.empty((B_FULL, T, 2 * H), np.float32)
    hT = np.empty((B_FULL, 2 * H), np.float32)
    cT = np.empty((B_FULL, 2 * H), np.float32)
    for core in range(N_CORES):
        fwd = core < 4
        q = core % 4
        rows = slice(q * B, (q + 1) * B)
        hs = unpack_hseq(res.results[core]["hseq"])
        if fwd:
            seq_out[rows, :, :H] = hs
        else:
            seq_out[rows, :, H:] = hs[:, ::-1]
        hcol = slice(0, H) if fwd else slice(H, 2 * H)
        hT[rows, hcol] = unpack_state(res.results[core]["hT"])
        cT[rows, hcol] = unpack_state(res.results[core]["cT"])
    return seq_out, hT, cT


def kernel(input_seq, h_0, c_0,
           Wih_f0, Whh_f0, b_f0, Wih_b0, Whh_b0, b_b0,
           Wih_f1, Whh_f1, b_f1, Wih_b1, Whh_b1, b_b1):
    input_seq = np.asarray(input_seq, np.float32)
    h_0 = np.asarray(h_0, np.float32)
    c_0 = np.asarray(c_0, np.float32)

    seq1, hT0, cT0 = _run_bilayer(
        input_seq, h_0[0], c_0[0], h_0[2], c_0[2],
        (np.asarray(Wih_f0, np.float32), np.asarray(Whh_f0, np.float32)),
        np.asarray(b_f0, np.float32),
        (np.asarray(Wih_b0, np.float32), np.asarray(Whh_b0, np.float32)),
        np.asarray(b_b0, np.float32))

    seq2, hT1, cT1 = _run_bilayer(
        seq1, h_0[1], c_0[1], h_0[3], c_0[3],
        (np.asarray(Wih_f1, np.float32), np.asarray(Whh_f1, np.float32)),
        np.asarray(b_f1, np.float32),
        (np.asarray(Wih_b1, np.float32), np.asarray(Whh_b1, np.float32)),
        np.asarray(b_b1, np.float32))

    h_n = np.stack([hT0, hT1])
    c_n = np.stack([cT0, cT1])
    return seq2, h_n, c_n


def build_layer_reps(Din, reps):
    """Timing variant: the whole T-loop body repeated `reps` times."""
    return build_layer(Din, reps=reps)
